# revision 9
# baseline (speedup 1.0000x reference)
"""MoE grouped linear (gmm) kernel for 8 Trainium2 NeuronCores.

Strategy (balanced 2D split: 2 K-halves x 4 out-quarters):
  - The expert-parallel baseline pads every core to the max group size, so
    the slowest core does up to maxg/mean extra PE work. Instead, EVERY
    core processes ALL T tokens over (IN/2) contraction rows and (OUT/4)
    output columns: exactly T*IN*OUT/8 MACs per core regardless of the
    group-size skew, and the instruction stream is identical across cores
    (group boundaries are the same for everyone) - clean SPMD.
  - Core c = (kh, oq) gets X^T[kh-half] and, for all E experts,
    W_e[kh-half, oq-quarter]. Weight HBM traffic is exactly one
    expert-equivalent per core (no replication); X is replicated 4x but
    only half-length. Everything streams as fp16 (host-converted,
    round-to-nearest), accumulates in fp32 PSUM.
  - Tokens stay sorted by expert; matmul token chunks split at the group
    boundaries (baked into the program from the actual group_sizes).
  - Each core writes its fp16 partial [OUT/4, T]; host sums the two
    K-half partials in fp32, adds the bias, and reassembles [T, OUT].
  - Input DMA is issued in compute-consumption order (chunk-major units,
    W sliced per (o-block, expert-pair)) so the PE starts as soon as the
    first ~1 MB lands and then never starves; a short HAM-warmup matmul
    burst on the last PSUM bank keeps the PE clock at 2.4 GHz without
    blocking the first real unit. Outputs gather 4 o-blocks per token
    chunk into one DMA to keep the semaphore count (and the end-of-
    program per-semaphore reset tail) small.
"""

import numpy as np

import concourse.bass as bass
from concourse import bacc
import concourse.mybir as mybir
import concourse.tile as tile
from concourse.bass_utils import run_bass_kernel_spmd

N_CORES = 8
KSPLIT = 2   # contraction-dim split factor
OSPLIT = 4   # output-dim split factor
P = 128
CHUNK = 512  # token-chunk (PSUM bank = 512 fp32)

_BUILD_CACHE: dict = {}


def _chunk_segments(groups, t_tokens):
    """Per 512-token chunk: (t0, clen, [(off_in_chunk, len, expert)])."""
    bounds = np.concatenate([[0], np.cumsum(np.asarray(groups, np.int64))])
    out = []
    t0 = 0
    while t0 < t_tokens:
        cl = min(CHUNK, t_tokens - t0)
        segs = []
        for e in range(len(groups)):
            s = max(t0, int(bounds[e]))
            t = min(t0 + cl, int(bounds[e + 1]))
            if t > s:
                segs.append((s - t0, t - s, e))
        out.append((t0, cl, segs))
        t0 += cl
    return out


def _build_program(t_tokens, n_in, n_out, n_exp, groups):
    kh = n_in // KSPLIT    # contraction rows per core
    kb = kh // P           # k-blocks
    oq = n_out // OSPLIT   # out cols per core
    ob = oq // P           # o-blocks
    chunks = _chunk_segments(groups, t_tokens)
    nch = len(chunks)
    f16 = mybir.dt.float16
    f32 = mybir.dt.float32

    # W DMA pieces: (o-block, expert-pair). npair pairs cover all experts.
    npair = (n_exp + 1) // 2

    nc = bacc.Bacc(
        "TRN2", target_bir_lowering=False, debug=False, num_devices=N_CORES
    )
    x = nc.dram_tensor("x", [nch, P, kb, CHUNK], f16, kind="ExternalInput")
    w = nc.dram_tensor("w", [P, ob, n_exp, kb, P], f16, kind="ExternalInput")
    y = nc.dram_tensor("y", [P, ob, t_tokens], f16, kind="ExternalOutput")

    # Input DMA order: x-chunk pieces and W (ob, expert-pair) pieces
    # interleaved so each chunk's weights land just before its x does.
    # x0 is split at its first segment boundary so the very first matmul
    # only waits for ~1 MB of DMA.
    pieces = []
    emitted_pairs = set()
    for c, (t0, cl, segs) in enumerate(chunks):
        need = [pr for pr in dict.fromkeys(e // 2 for (_o, _l, e) in segs)
                if pr not in emitted_pairs]
        emitted_pairs.update(need)
        first_pair = segs[0][2] // 2 if segs else None
        if c == 0:
            xs = segs[0][1] if len(segs) > 1 else cl
            pieces.append(("x", 0, 0, xs))
            for pr in need:
                pieces.append(("w", 0, pr, None))
            if xs < cl:
                pieces.append(("x", 0, xs, cl))
            for oi in range(1, ob):
                for pr in need:
                    pieces.append(("w", oi, pr, None))
        elif need and first_pair in need:
            # chunk opens with a new expert-pair: o-block 0's weights
            # must precede the x piece, the rest follow it
            for pr in need:
                pieces.append(("w", 0, pr, None))
            pieces.append(("x", c, 0, cl))
            for oi in range(1, ob):
                for pr in need:
                    pieces.append(("w", oi, pr, None))
        else:
            # new pair (if any) only feeds a later segment; x first so the
            # chunk's opening segment can start the moment it lands
            pieces.append(("x", c, 0, cl))
            for oi in range(ob):
                for pr in need:
                    pieces.append(("w", oi, pr, None))

    with tile.TileContext(nc) as tc:
        with (
            tc.tile_pool(name="const", bufs=1) as constp,
            tc.tile_pool(name="xsb", bufs=1) as xp,
            tc.tile_pool(name="wsb", bufs=1) as wp,
            tc.tile_pool(name="outsb", bufs=3) as outp,
            tc.tile_pool(name="psum", bufs=1, space="PSUM") as psump,
        ):
            x_t = {}
            w_t = {}
            for kind, a, b, cend in pieces:
                if kind == "x":
                    if a not in x_t:
                        x_t[a] = xp.tile([P, kb, chunks[a][1]], f16,
                                         tag=f"x{a}", name=f"x{a}")
                    nc.sync.dma_start(
                        x_t[a][:, :, b:cend], x[a][:, :, b:cend]
                    )
                else:
                    e0 = b * 2
                    e1 = min(n_exp, e0 + 2)
                    w_t[(a, b)] = wp.tile([P, e1 - e0, kb, P], f16,
                                          tag=f"w{a}_{b}", name=f"w{a}_{b}")
                    nc.sync.dma_start(w_t[(a, b)][:], w[:, a, e0:e1])

            # HAM warmup: dependency-free matmuls while the prologue DMAs
            # stream, sized to end roughly when the first real data lands.
            # Uses the last PSUM bank so the first real units are not
            # blocked behind it.
            warm = constp.tile([P, CHUNK], f16)
            nc.gpsimd.memset(warm[:], 0)
            ps_warm = psump.tile([P, CHUNK], f32, tag="b7", name="warmps")
            for i in range(24):
                nc.tensor.matmul(
                    ps_warm[:], warm[:, :P], warm[:],
                    start=(i == 0), stop=(i == 23),
                )

            # Chunk-major units: all o-blocks of a token chunk back to
            # back, gathered into one output tile -> one DMA per chunk.
            ui = 0
            for c, (t0, cl, segs) in enumerate(chunks):
                ot = outp.tile([P, ob, cl], f16, tag="ot", name=f"ot{c}")
                for oi in range(ob):
                    ps = psump.tile([P, cl], f32, tag=f"b{ui % 8}",
                                    name=f"ps{oi}_{c}")
                    for (off, ln, e) in segs:
                        wt = w_t[(oi, e // 2)]
                        for k in range(kb):
                            nc.tensor.matmul(
                                ps[:, off:off + ln],
                                wt[:, e % 2, k, :],
                                x_t[c][:, k, off:off + ln],
                                start=(k == 0), stop=(k == kb - 1),
                            )
                    if (c + oi) % 2 == 0:
                        nc.scalar.activation(
                            ot[:, oi, :], ps[:],
                            mybir.ActivationFunctionType.Identity,
                        )
                    else:
                        nc.vector.tensor_copy(ot[:, oi, :], ps[:])
                    if c == nch - 1:
                        # last chunk: one small DMA per o-block right after
                        # its evacuation, to shorten the end-of-kernel drain
                        nc.scalar.dma_start(
                            y[:, oi, t0:t0 + cl], ot[:, oi, :]
                        )
                    ui += 1
                if c != nch - 1:
                    nc.scalar.dma_start(y[:, :, t0:t0 + cl], ot[:])
    nc.finalize()
    return nc


def _prepare(inputs, weight, bias, group_sizes):
    """Build (or reuse) the program and the per-core input maps."""
    inputs = np.asarray(inputs, dtype=np.float32)
    weight = np.asarray(weight, dtype=np.float32)
    bias = np.asarray(bias, dtype=np.float32)
    g = np.asarray(group_sizes).astype(np.int64)

    t_tokens, n_in = inputs.shape
    n_exp, _, n_out = weight.shape
    assert n_in % (KSPLIT * P) == 0 and n_out % (OSPLIT * P) == 0
    assert int(g.sum()) == t_tokens, "group_sizes must sum to token count"

    key = (t_tokens, n_in, n_out, n_exp, tuple(int(v) for v in g))
    if key not in _BUILD_CACHE:
        _BUILD_CACHE[key] = _build_program(
            t_tokens, n_in, n_out, n_exp, tuple(int(v) for v in g)
        )
    nc = _BUILD_CACHE[key]

    kh = n_in // KSPLIT
    kb = kh // P
    oq = n_out // OSPLIT
    ob = oq // P
    nch = (t_tokens + CHUNK - 1) // CHUNK
    tp = nch * CHUNK

    # X^T in fp16, chunk-major: [nch, P, kb, CHUNK] per K-half.
    xt = np.zeros((n_in, tp), np.float16)
    xt[:, :t_tokens] = inputs.T.astype(np.float16)
    x_halves = []
    for khi in range(KSPLIT):
        sl = xt[khi * kh:(khi + 1) * kh]                  # [kh, tp]
        sl = sl.reshape(kb, P, nch, CHUNK).transpose(2, 1, 0, 3)
        x_halves.append(np.ascontiguousarray(sl))         # [nch, P, kb, CHUNK]

    w16 = weight.astype(np.float16)                       # [E, n_in, n_out]
    in_maps = []
    for c in range(N_CORES):
        khi, oqi = c // OSPLIT, c % OSPLIT
        wsl = w16[:, khi * kh:(khi + 1) * kh, oqi * oq:(oqi + 1) * oq]
        # [E, kh, oq] -> [P(k in block), ob, E, kb, P(o)]
        wsl = wsl.reshape(n_exp, kb, P, ob, P).transpose(2, 3, 0, 1, 4)
        in_maps.append({
            "x": x_halves[khi],
            "w": np.ascontiguousarray(wsl),
        })
    return nc, in_maps, g, (t_tokens, n_out), (ob, oq)


def kernel(inputs, weight, bias, group_sizes):
    nc, in_maps, g, (t_tokens, n_out), (ob, oq) = _prepare(
        inputs, weight, bias, group_sizes
    )
    res = run_bass_kernel_spmd(nc, in_maps, core_ids=list(range(N_CORES)))

    bias = np.asarray(bias, dtype=np.float32)
    out = np.empty((t_tokens, n_out), np.float32)
    for oqi in range(OSPLIT):
        acc = res.results[oqi]["y"].astype(np.float32)
        for khi in range(1, KSPLIT):
            acc += res.results[khi * OSPLIT + oqi]["y"].astype(np.float32)
        # y is [P, ob, T]; out column within the quarter = ob*P + p
        out[:, oqi * oq:(oqi + 1) * oq] = (
            acc.transpose(1, 0, 2).reshape(oq, t_tokens).T
        )
    out += bias[None, :]
    return out


# revision 10
# speedup vs baseline: 1.0379x; 1.0379x over previous
"""MoE grouped linear (gmm) kernel for 8 Trainium2 NeuronCores.

Strategy (balanced 2D split: 2 K-halves x 4 out-quarters):
  - The expert-parallel baseline pads every core to the max group size, so
    the slowest core does up to maxg/mean extra PE work. Instead, EVERY
    core processes ALL T tokens over (IN/2) contraction rows and (OUT/4)
    output columns: exactly T*IN*OUT/8 MACs per core regardless of the
    group-size skew, and the instruction stream is identical across cores
    (group boundaries are the same for everyone) - clean SPMD.
  - Core c = (kh, oq) gets X^T[kh-half] and, for all E experts,
    W_e[kh-half, oq-quarter]. Weight HBM traffic is exactly one
    expert-equivalent per core (no replication); X is replicated 4x but
    only half-length. Everything streams as fp16 (host-converted,
    round-to-nearest), accumulates in fp32 PSUM.
  - Tokens stay sorted by expert; matmul token chunks split at the group
    boundaries (baked into the program from the actual group_sizes).
  - Each core writes its fp16 partial [OUT/4, T]; host sums the two
    K-half partials in fp32, adds the bias, and reassembles [T, OUT].
  - Input DMA is issued in compute-consumption order (chunk-major units,
    W sliced per (o-block, expert-pair)) so the PE starts as soon as the
    first ~1 MB lands and then never starves; a short HAM-warmup matmul
    burst on the last PSUM bank keeps the PE clock at 2.4 GHz without
    blocking the first real unit. Outputs gather 4 o-blocks per token
    chunk into one DMA to keep the semaphore count (and the end-of-
    program per-semaphore reset tail) small.
"""

import numpy as np

import concourse.bass as bass
from concourse import bacc
import concourse.mybir as mybir
import concourse.tile as tile
from concourse.bass_utils import run_bass_kernel_spmd

N_CORES = 8
KSPLIT = 2   # contraction-dim split factor
OSPLIT = 4   # output-dim split factor
P = 128
CHUNK = 512  # token-chunk (PSUM bank = 512 fp32)

_BUILD_CACHE: dict = {}


def _chunk_segments(groups, t_tokens):
    """Per 512-token chunk: (t0, clen, [(off_in_chunk, len, expert)])."""
    bounds = np.concatenate([[0], np.cumsum(np.asarray(groups, np.int64))])
    out = []
    t0 = 0
    while t0 < t_tokens:
        cl = min(CHUNK, t_tokens - t0)
        segs = []
        for e in range(len(groups)):
            s = max(t0, int(bounds[e]))
            t = min(t0 + cl, int(bounds[e + 1]))
            if t > s:
                segs.append((s - t0, t - s, e))
        out.append((t0, cl, segs))
        t0 += cl
    return out


def _build_program(t_tokens, n_in, n_out, n_exp, groups):
    kh = n_in // KSPLIT    # contraction rows per core
    kb = kh // P           # k-blocks
    oq = n_out // OSPLIT   # out cols per core
    ob = oq // P           # o-blocks
    chunks = _chunk_segments(groups, t_tokens)
    nch = len(chunks)
    f16 = mybir.dt.float16
    f32 = mybir.dt.float32

    # W DMA pieces: (o-block, expert-pair). npair pairs cover all experts.
    npair = (n_exp + 1) // 2

    nc = bacc.Bacc(
        "TRN2", target_bir_lowering=False, debug=False, num_devices=N_CORES
    )
    x = nc.dram_tensor("x", [nch, P, kb, CHUNK], f16, kind="ExternalInput")
    w = nc.dram_tensor("w", [P, ob, n_exp, kb, P], f16, kind="ExternalInput")
    y = nc.dram_tensor("y", [P, ob, t_tokens], f16, kind="ExternalOutput")

    # Input DMA order: x-chunk pieces and W (ob, expert-pair) pieces
    # interleaved so each chunk's weights land just before its x does.
    # x0 is split at its first segment boundary so the very first matmul
    # only waits for ~1 MB of DMA.
    pieces = []
    emitted_pairs = set()
    for c, (t0, cl, segs) in enumerate(chunks):
        need = [pr for pr in dict.fromkeys(e // 2 for (_o, _l, e) in segs)
                if pr not in emitted_pairs]
        emitted_pairs.update(need)
        first_pair = segs[0][2] // 2 if segs else None
        if c == 0:
            xs = segs[0][1] if len(segs) > 1 else cl
            pieces.append(("x", 0, 0, xs))
            for pr in need:
                pieces.append(("w", 0, pr, None))
            if xs < cl:
                pieces.append(("x", 0, xs, cl))
            for oi in range(1, ob):
                for pr in need:
                    pieces.append(("w", oi, pr, None))
        elif need and first_pair in need:
            # chunk opens with a new expert-pair: o-block 0's weights
            # must precede the x piece, the rest follow it
            for pr in need:
                pieces.append(("w", 0, pr, None))
            pieces.append(("x", c, 0, cl))
            for oi in range(1, ob):
                for pr in need:
                    pieces.append(("w", oi, pr, None))
        else:
            # new pair (if any) only feeds a later segment; x first so the
            # chunk's opening segment can start the moment it lands
            pieces.append(("x", c, 0, cl))
            for oi in range(ob):
                for pr in need:
                    pieces.append(("w", oi, pr, None))

    with tile.TileContext(nc) as tc:
        with (
            tc.tile_pool(name="const", bufs=1) as constp,
            tc.tile_pool(name="xsb", bufs=1) as xp,
            tc.tile_pool(name="wsb", bufs=1) as wp,
            tc.tile_pool(name="outsb", bufs=3) as outp,
            tc.tile_pool(name="psum", bufs=1, space="PSUM") as psump,
        ):
            x_t = {}
            w_t = {}
            for kind, a, b, cend in pieces:
                if kind == "x":
                    if a not in x_t:
                        x_t[a] = xp.tile([P, kb, chunks[a][1]], f16,
                                         tag=f"x{a}", name=f"x{a}")
                    nc.sync.dma_start(
                        x_t[a][:, :, b:cend], x[a][:, :, b:cend]
                    )
                else:
                    e0 = b * 2
                    e1 = min(n_exp, e0 + 2)
                    w_t[(a, b)] = wp.tile([P, e1 - e0, kb, P], f16,
                                          tag=f"w{a}_{b}", name=f"w{a}_{b}")
                    nc.sync.dma_start(w_t[(a, b)][:], w[:, a, e0:e1])

            # HAM warmup: dependency-free matmuls while the prologue DMAs
            # stream, sized to end roughly when the first real data lands.
            # Uses the last PSUM bank so the first real units are not
            # blocked behind it.
            warm = constp.tile([P, CHUNK], f16)
            nc.gpsimd.memset(warm[:], 0)
            ps_warm = psump.tile([P, CHUNK], f32, tag="b7", name="warmps")
            for i in range(12):
                nc.tensor.matmul(
                    ps_warm[:], warm[:, :P], warm[:],
                    start=(i == 0), stop=(i == 11),
                )

            # Chunk-major units: all o-blocks of a token chunk back to
            # back, gathered into one output tile -> one DMA per chunk.
            ui = 0
            for c, (t0, cl, segs) in enumerate(chunks):
                ot = outp.tile([P, ob, cl], f16, tag="ot", name=f"ot{c}")
                for oi in range(ob):
                    ps = psump.tile([P, cl], f32, tag=f"b{ui % 8}",
                                    name=f"ps{oi}_{c}")
                    for (off, ln, e) in segs:
                        wt = w_t[(oi, e // 2)]
                        for k in range(kb):
                            nc.tensor.matmul(
                                ps[:, off:off + ln],
                                wt[:, e % 2, k, :],
                                x_t[c][:, k, off:off + ln],
                                start=(k == 0), stop=(k == kb - 1),
                            )
                    if (c + oi) % 2 == 0:
                        nc.scalar.activation(
                            ot[:, oi, :], ps[:],
                            mybir.ActivationFunctionType.Identity,
                        )
                    else:
                        nc.vector.tensor_copy(ot[:, oi, :], ps[:])
                    if c == nch - 1:
                        # last chunk: one small DMA per o-block right after
                        # its evacuation, to shorten the end-of-kernel drain
                        nc.gpsimd.dma_start(
                            y[:, oi, t0:t0 + cl], ot[:, oi, :]
                        )
                    ui += 1
                if c != nch - 1:
                    # SWDGE (gpsimd) keeps output DMAs off the HWDGE
                    # semaphore lanes, so input-piece issue never chains
                    # behind an output that waits on compute
                    nc.gpsimd.dma_start(y[:, :, t0:t0 + cl], ot[:])
    nc.finalize()
    return nc


def _prepare(inputs, weight, bias, group_sizes):
    """Build (or reuse) the program and the per-core input maps."""
    inputs = np.asarray(inputs, dtype=np.float32)
    weight = np.asarray(weight, dtype=np.float32)
    bias = np.asarray(bias, dtype=np.float32)
    g = np.asarray(group_sizes).astype(np.int64)

    t_tokens, n_in = inputs.shape
    n_exp, _, n_out = weight.shape
    assert n_in % (KSPLIT * P) == 0 and n_out % (OSPLIT * P) == 0
    assert int(g.sum()) == t_tokens, "group_sizes must sum to token count"

    key = (t_tokens, n_in, n_out, n_exp, tuple(int(v) for v in g))
    if key not in _BUILD_CACHE:
        _BUILD_CACHE[key] = _build_program(
            t_tokens, n_in, n_out, n_exp, tuple(int(v) for v in g)
        )
    nc = _BUILD_CACHE[key]

    kh = n_in // KSPLIT
    kb = kh // P
    oq = n_out // OSPLIT
    ob = oq // P
    nch = (t_tokens + CHUNK - 1) // CHUNK
    tp = nch * CHUNK

    # X^T in fp16, chunk-major: [nch, P, kb, CHUNK] per K-half.
    xt = np.zeros((n_in, tp), np.float16)
    xt[:, :t_tokens] = inputs.T.astype(np.float16)
    x_halves = []
    for khi in range(KSPLIT):
        sl = xt[khi * kh:(khi + 1) * kh]                  # [kh, tp]
        sl = sl.reshape(kb, P, nch, CHUNK).transpose(2, 1, 0, 3)
        x_halves.append(np.ascontiguousarray(sl))         # [nch, P, kb, CHUNK]

    w16 = weight.astype(np.float16)                       # [E, n_in, n_out]
    in_maps = []
    for c in range(N_CORES):
        khi, oqi = c // OSPLIT, c % OSPLIT
        wsl = w16[:, khi * kh:(khi + 1) * kh, oqi * oq:(oqi + 1) * oq]
        # [E, kh, oq] -> [P(k in block), ob, E, kb, P(o)]
        wsl = wsl.reshape(n_exp, kb, P, ob, P).transpose(2, 3, 0, 1, 4)
        in_maps.append({
            "x": x_halves[khi],
            "w": np.ascontiguousarray(wsl),
        })
    return nc, in_maps, g, (t_tokens, n_out), (ob, oq)


def kernel(inputs, weight, bias, group_sizes):
    nc, in_maps, g, (t_tokens, n_out), (ob, oq) = _prepare(
        inputs, weight, bias, group_sizes
    )
    res = run_bass_kernel_spmd(nc, in_maps, core_ids=list(range(N_CORES)))

    bias = np.asarray(bias, dtype=np.float32)
    out = np.empty((t_tokens, n_out), np.float32)
    for oqi in range(OSPLIT):
        acc = res.results[oqi]["y"].astype(np.float32)
        for khi in range(1, KSPLIT):
            acc += res.results[khi * OSPLIT + oqi]["y"].astype(np.float32)
        # y is [P, ob, T]; out column within the quarter = ob*P + p
        out[:, oqi * oq:(oqi + 1) * oq] = (
            acc.transpose(1, 0, 2).reshape(oq, t_tokens).T
        )
    out += bias[None, :]
    return out


# revision 11
# speedup vs baseline: 1.0394x; 1.0014x over previous
"""MoE grouped linear (gmm) kernel for 8 Trainium2 NeuronCores.

Strategy (balanced 2D split: 2 K-halves x 4 out-quarters):
  - The expert-parallel baseline pads every core to the max group size, so
    the slowest core does up to maxg/mean extra PE work. Instead, EVERY
    core processes ALL T tokens over (IN/2) contraction rows and (OUT/4)
    output columns: exactly T*IN*OUT/8 MACs per core regardless of the
    group-size skew, and the instruction stream is identical across cores
    (group boundaries are the same for everyone) - clean SPMD.
  - Core c = (kh, oq) gets X^T[kh-half] and, for all E experts,
    W_e[kh-half, oq-quarter]. Weight HBM traffic is exactly one
    expert-equivalent per core (no replication); X is replicated 4x but
    only half-length. Everything streams as fp16 (host-converted,
    round-to-nearest), accumulates in fp32 PSUM.
  - Tokens stay sorted by expert; matmul token chunks split at the group
    boundaries (baked into the program from the actual group_sizes).
  - Each core writes its fp16 partial [OUT/4, T]; host sums the two
    K-half partials in fp32, adds the bias, and reassembles [T, OUT].
  - Input DMA is issued in compute-consumption order (chunk-major units,
    W sliced per (o-block, expert-pair)) so the PE starts as soon as the
    first ~1 MB lands and then never starves; a short HAM-warmup matmul
    burst on the last PSUM bank keeps the PE clock at 2.4 GHz without
    blocking the first real unit. Outputs gather 4 o-blocks per token
    chunk into one DMA to keep the semaphore count (and the end-of-
    program per-semaphore reset tail) small.
"""

import numpy as np

import concourse.bass as bass
from concourse import bacc
import concourse.mybir as mybir
import concourse.tile as tile
from concourse.bass_utils import run_bass_kernel_spmd

N_CORES = 8
KSPLIT = 2   # contraction-dim split factor
OSPLIT = 4   # output-dim split factor
P = 128
CHUNK = 512  # token-chunk (PSUM bank = 512 fp32)

_BUILD_CACHE: dict = {}


def _chunk_segments(groups, t_tokens):
    """Per 512-token chunk: (t0, clen, [(off_in_chunk, len, expert)])."""
    bounds = np.concatenate([[0], np.cumsum(np.asarray(groups, np.int64))])
    out = []
    t0 = 0
    while t0 < t_tokens:
        cl = min(CHUNK, t_tokens - t0)
        segs = []
        for e in range(len(groups)):
            s = max(t0, int(bounds[e]))
            t = min(t0 + cl, int(bounds[e + 1]))
            if t > s:
                segs.append((s - t0, t - s, e))
        out.append((t0, cl, segs))
        t0 += cl
    return out


def _build_program(t_tokens, n_in, n_out, n_exp, groups):
    kh = n_in // KSPLIT    # contraction rows per core
    kb = kh // P           # k-blocks
    oq = n_out // OSPLIT   # out cols per core
    ob = oq // P           # o-blocks
    chunks = _chunk_segments(groups, t_tokens)
    nch = len(chunks)
    f16 = mybir.dt.float16
    f32 = mybir.dt.float32

    # W DMA pieces: (o-block, expert-pair). npair pairs cover all experts.
    npair = (n_exp + 1) // 2

    nc = bacc.Bacc(
        "TRN2", target_bir_lowering=False, debug=False, num_devices=N_CORES
    )
    x = nc.dram_tensor("x", [nch, P, kb, CHUNK], f16, kind="ExternalInput")
    w = nc.dram_tensor("w", [P, ob, n_exp, kb, P], f16, kind="ExternalInput")
    y = nc.dram_tensor("y", [P, ob, t_tokens], f16, kind="ExternalOutput")

    # Input DMA order: x-chunk pieces and W (ob, expert-pair) pieces
    # interleaved so each chunk's weights land just before its x does.
    # x0 is split at its first segment boundary so the very first matmul
    # only waits for ~1 MB of DMA.
    pieces = []
    emitted_pairs = set()
    for c, (t0, cl, segs) in enumerate(chunks):
        need = [pr for pr in dict.fromkeys(e // 2 for (_o, _l, e) in segs)
                if pr not in emitted_pairs]
        emitted_pairs.update(need)
        first_pair = segs[0][2] // 2 if segs else None
        if c == 0:
            xs = segs[0][1] if len(segs) > 1 else cl
            pieces.append(("x", 0, 0, xs))
            for pr in need:
                pieces.append(("w", 0, pr, None))
            if xs < cl:
                pieces.append(("x", 0, xs, cl))
            for oi in range(1, ob):
                for pr in need:
                    pieces.append(("w", oi, pr, None))
        elif need and first_pair in need:
            # chunk opens with a new expert-pair: o-block 0's weights
            # must precede the x piece, the rest follow it
            for pr in need:
                pieces.append(("w", 0, pr, None))
            pieces.append(("x", c, 0, cl))
            for oi in range(1, ob):
                for pr in need:
                    pieces.append(("w", oi, pr, None))
        else:
            # new pair (if any) only feeds a later segment; x first so the
            # chunk's opening segment can start the moment it lands
            pieces.append(("x", c, 0, cl))
            for oi in range(ob):
                for pr in need:
                    pieces.append(("w", oi, pr, None))

    with tile.TileContext(nc) as tc:
        with (
            tc.tile_pool(name="const", bufs=1) as constp,
            tc.tile_pool(name="xsb", bufs=1) as xp,
            tc.tile_pool(name="wsb", bufs=1) as wp,
            tc.tile_pool(name="outsb", bufs=3) as outp,
            tc.tile_pool(name="psum", bufs=1, space="PSUM") as psump,
        ):
            x_t = {}
            w_t = {}
            for kind, a, b, cend in pieces:
                if kind == "x":
                    if a not in x_t:
                        x_t[a] = xp.tile([P, kb, chunks[a][1]], f16,
                                         tag=f"x{a}", name=f"x{a}")
                    nc.sync.dma_start(
                        x_t[a][:, :, b:cend], x[a][:, :, b:cend]
                    )
                else:
                    e0 = b * 2
                    e1 = min(n_exp, e0 + 2)
                    w_t[(a, b)] = wp.tile([P, e1 - e0, kb, P], f16,
                                          tag=f"w{a}_{b}", name=f"w{a}_{b}")
                    nc.sync.dma_start(w_t[(a, b)][:], w[:, a, e0:e1])

            # HAM warmup: dependency-free matmuls while the prologue DMAs
            # stream, sized to end roughly when the first real data lands.
            # Uses the last PSUM bank so the first real units are not
            # blocked behind it.
            warm = constp.tile([P, CHUNK], f16)
            nc.gpsimd.memset(warm[:], 0)
            ps_warm = psump.tile([P, CHUNK], f32, tag="b7", name="warmps")
            for i in range(18):
                nc.tensor.matmul(
                    ps_warm[:], warm[:, :P], warm[:],
                    start=(i == 0), stop=(i == 17),
                )

            # Chunk-major units: all o-blocks of a token chunk back to
            # back, gathered into one output tile -> one DMA per chunk.
            ui = 0
            for c, (t0, cl, segs) in enumerate(chunks):
                ot = outp.tile([P, ob, cl], f16, tag="ot", name=f"ot{c}")
                for oi in range(ob):
                    ps = psump.tile([P, cl], f32, tag=f"b{ui % 8}",
                                    name=f"ps{oi}_{c}")
                    for (off, ln, e) in segs:
                        wt = w_t[(oi, e // 2)]
                        for k in range(kb):
                            nc.tensor.matmul(
                                ps[:, off:off + ln],
                                wt[:, e % 2, k, :],
                                x_t[c][:, k, off:off + ln],
                                start=(k == 0), stop=(k == kb - 1),
                            )
                    if (c + oi) % 2 == 0:
                        nc.scalar.activation(
                            ot[:, oi, :], ps[:],
                            mybir.ActivationFunctionType.Identity,
                        )
                    else:
                        nc.vector.tensor_copy(ot[:, oi, :], ps[:])
                    if c == nch - 1:
                        # last chunk: one small DMA per o-block right after
                        # its evacuation, to shorten the end-of-kernel drain
                        nc.gpsimd.dma_start(
                            y[:, oi, t0:t0 + cl], ot[:, oi, :]
                        )
                    ui += 1
                if c != nch - 1:
                    # SWDGE (gpsimd) keeps output DMAs off the HWDGE
                    # semaphore lanes, so input-piece issue never chains
                    # behind an output that waits on compute
                    nc.gpsimd.dma_start(y[:, :, t0:t0 + cl], ot[:])
    nc.finalize()
    return nc


def _prepare(inputs, weight, bias, group_sizes):
    """Build (or reuse) the program and the per-core input maps."""
    inputs = np.asarray(inputs, dtype=np.float32)
    weight = np.asarray(weight, dtype=np.float32)
    bias = np.asarray(bias, dtype=np.float32)
    g = np.asarray(group_sizes).astype(np.int64)

    t_tokens, n_in = inputs.shape
    n_exp, _, n_out = weight.shape
    assert n_in % (KSPLIT * P) == 0 and n_out % (OSPLIT * P) == 0
    assert int(g.sum()) == t_tokens, "group_sizes must sum to token count"

    key = (t_tokens, n_in, n_out, n_exp, tuple(int(v) for v in g))
    if key not in _BUILD_CACHE:
        _BUILD_CACHE[key] = _build_program(
            t_tokens, n_in, n_out, n_exp, tuple(int(v) for v in g)
        )
    nc = _BUILD_CACHE[key]

    kh = n_in // KSPLIT
    kb = kh // P
    oq = n_out // OSPLIT
    ob = oq // P
    nch = (t_tokens + CHUNK - 1) // CHUNK
    tp = nch * CHUNK

    # X^T in fp16, chunk-major: [nch, P, kb, CHUNK] per K-half.
    xt = np.zeros((n_in, tp), np.float16)
    xt[:, :t_tokens] = inputs.T.astype(np.float16)
    x_halves = []
    for khi in range(KSPLIT):
        sl = xt[khi * kh:(khi + 1) * kh]                  # [kh, tp]
        sl = sl.reshape(kb, P, nch, CHUNK).transpose(2, 1, 0, 3)
        x_halves.append(np.ascontiguousarray(sl))         # [nch, P, kb, CHUNK]

    w16 = weight.astype(np.float16)                       # [E, n_in, n_out]
    in_maps = []
    for c in range(N_CORES):
        khi, oqi = c // OSPLIT, c % OSPLIT
        wsl = w16[:, khi * kh:(khi + 1) * kh, oqi * oq:(oqi + 1) * oq]
        # [E, kh, oq] -> [P(k in block), ob, E, kb, P(o)]
        wsl = wsl.reshape(n_exp, kb, P, ob, P).transpose(2, 3, 0, 1, 4)
        in_maps.append({
            "x": x_halves[khi],
            "w": np.ascontiguousarray(wsl),
        })
    return nc, in_maps, g, (t_tokens, n_out), (ob, oq)


def kernel(inputs, weight, bias, group_sizes):
    nc, in_maps, g, (t_tokens, n_out), (ob, oq) = _prepare(
        inputs, weight, bias, group_sizes
    )
    res = run_bass_kernel_spmd(nc, in_maps, core_ids=list(range(N_CORES)))

    bias = np.asarray(bias, dtype=np.float32)
    out = np.empty((t_tokens, n_out), np.float32)
    for oqi in range(OSPLIT):
        acc = res.results[oqi]["y"].astype(np.float32)
        for khi in range(1, KSPLIT):
            acc += res.results[khi * OSPLIT + oqi]["y"].astype(np.float32)
        # y is [P, ob, T]; out column within the quarter = ob*P + p
        out[:, oqi * oq:(oqi + 1) * oq] = (
            acc.transpose(1, 0, 2).reshape(oq, t_tokens).T
        )
    out += bias[None, :]
    return out
